# revision 19
# baseline (speedup 1.0000x reference)
"""Trainium2 Bass kernel for nn_AttentionLayer (DIN-style attention MLP).

Per batch row b (B=2048, T=200, D=64, H1=80, H2=40):
  info = [q, k, q-k, q*k];  y1 = info @ W1 + b1
       = k @ (W1b-W1c) + (q*k) @ W1d + (q @ (W1a+W1c) + b1)
  h1 = prelu(y1, a1);  y2 = h1 @ W2 + b2;  h2 = prelu(y2, a2)
  logits = h2 @ Wf;  w = softmax(logits);  out = w @ v

Pure data-parallel across 8 cores (256 batches each). v2 layout:
- 8 super-chunks of 32 batches (6400 rows) per core.
- k: SWDGE cast f32->f16 -> HWDGE xbar transpose (alternating SP/Act
  rings) -> SWDGE dup to partitions 64-127 -> ONE DVE tensor_tensor
  multiply against a host-marshalled broadcast of q (qrep, f16 input)
  forms (q*k).T in place. mm1 is a W-stationary f16 stream (N=400).
- PReLU: fused custom DVE op (select + per-batch bias via PageIdx).
- logits: Wf packed into zero-padded [104,32] stationaries so 16
  matmuls/super-chunk accumulate logits for all 32 batches into one
  [32,200] PSUM tile (batch-major).
- softmax: Act exp with fused row-sum accumulator, DVE reciprocal +
  scale; weights transposed to t-major via 2 PE transposes.
- w@v: w-stationary matmuls (LDWEIGHTS = 1 column) with
  tile_position col-quadrants packing 4 batches/PSUM-partition-group;
  one [128,512] PSUM bank holds all 32 outputs; Act copies to SBUF and
  a strided DMA writes [32,64] rows straight to HBM.
"""

import numpy as np

B, T, D = 2048, 200, 64
H1, H2 = 80, 40
N_CORES = 8
BC = B // N_CORES          # 256 batches per core
S2B = 32                   # batches per super-chunk
N_S2 = BC // S2B           # 8 super-chunks
S2_ROWS = S2B * T          # 6400 rows
N_BLK = S2_ROWS // 128     # 50 transpose blocks
CH_B = 2                   # batches per chunk
CH_ROWS = CH_B * T         # 400
N_CH = S2B // CH_B         # 16 chunks per super-chunk

_cache = {}


def _register_prelu_op():
    import concourse.dve_ops as dve_ops
    from concourse.dve_ops import DveOp, OPS
    from concourse.dve_spec import Spec, Src0, Src1, C0, C1, Zero, select, lower, PageIdx
    from concourse.dve_uop import DveOpSpec

    if "PRELU_PG_ANT" in dve_ops._SUB_OPCODE_FOR_NAME:
        for op in OPS:
            if op.name == "PRELU_PG_ANT":
                return op
    t = Src0 + PageIdx(C0, C1)
    spec = Spec(body=select(t > Zero, t, t * Src1))
    opcode = dve_ops._CUSTOM_DVE_ROW_BASE + len(OPS)
    shas = {}
    for ver in ("v3", "v4"):
        r = DveOpSpec(name="PRELU_PG_ANT", opcode=opcode,
                      uops=lower(spec, ver=ver), rd1_en=True)
        shas[ver] = r.sha(ver)
    op = DveOp("PRELU_PG_ANT", spec, subdim=True, uops_sha=shas)
    OPS.append(op)
    dve_ops._SUB_OPCODE_FOR_NAME["PRELU_PG_ANT"] = opcode
    return op


def _build():
    if "nc" in _cache:
        return _cache["nc"]
    import concourse.bacc as bacc
    import concourse.mybir as mybir
    import concourse.tile as tile
    from concourse import masks
    from concourse.ap import AP

    PRELU = _register_prelu_op()
    f16 = mybir.dt.float16
    f32 = mybir.dt.float32
    AF = mybir.ActivationFunctionType
    ALU = mybir.AluOpType

    nc = bacc.Bacc("TRN2", target_bir_lowering=False, debug=False,
                   num_devices=N_CORES)

    q_d = nc.dram_tensor("q", [BC, D], f32, kind="ExternalInput")
    k_d = nc.dram_tensor("k", [BC, T, D], f32, kind="ExternalInput")
    v_d = nc.dram_tensor("v", [BC, T, D], f32, kind="ExternalInput")
    qrep_d = nc.dram_tensor("qrep", [D, BC * T], f16, kind="ExternalInput")
    w1s_d = nc.dram_tensor("w1s", [128, H1], f16, kind="ExternalInput")
    w1ac_d = nc.dram_tensor("w1ac", [D, H1], f32, kind="ExternalInput")
    b1_d = nc.dram_tensor("b1", [H1], f32, kind="ExternalInput")
    w2s_d = nc.dram_tensor("w2s", [H1, H2], f16, kind="ExternalInput")
    b2c_d = nc.dram_tensor("b2c", [104], f32, kind="ExternalInput")
    a1r_d = nc.dram_tensor("a1r", [H1, CH_ROWS], f16, kind="ExternalInput")
    a2r_d = nc.dram_tensor("a2r", [104, CH_ROWS], f16, kind="ExternalInput")
    smeg_d = nc.dram_tensor("smeg", [104, 16 * S2B], f16, kind="ExternalInput")
    out_d = nc.dram_tensor("out", [BC, D], f32, kind="ExternalOutput")

    kv = k_d.ap().rearrange("b t d -> (b t) d").rearrange("(n p) d -> p n d", p=128)

    with tile.TileContext(nc) as tc:
        with (
            tc.tile_pool(name="const", bufs=1) as cpool,
            tc.tile_pool(name="kb", bufs=2) as kbpool,
            tc.tile_pool(name="kt", bufs=2) as ktpool,
            tc.tile_pool(name="qr", bufs=2) as qrpool,
            tc.tile_pool(name="vb", bufs=2) as vbpool,
            tc.tile_pool(name="h", bufs=4) as hpool,
            tc.tile_pool(name="sm", bufs=2) as smpool,
            tc.tile_pool(name="p1", bufs=2, space="PSUM") as p1pool,
            tc.tile_pool(name="p2", bufs=2, space="PSUM") as p2pool,
            tc.tile_pool(name="pL", bufs=1, space="PSUM") as pLpool,
            tc.tile_pool(name="pv", bufs=2, space="PSUM") as pvpool,
            tc.tile_pool(name="pm", bufs=1, space="PSUM") as pmpool,
        ):
            # ======== one-time setup ========
            ident32 = cpool.tile([128, 128], f32)
            masks.make_identity(nc, ident32[:])
            ident16 = cpool.tile([64, 64], f16)
            nc.vector.tensor_copy(ident16[:], ident32[0:64, 0:64])

            w1s16 = cpool.tile([128, H1], f16)
            nc.sync.dma_start(w1s16[:], w1s_d.ap())
            w1ac = cpool.tile([D, H1], f32)
            nc.sync.dma_start(w1ac[:], w1ac_d.ap())
            w2s16 = cpool.tile([H1, H2], f16)
            nc.sync.dma_start(w2s16[:], w2s_d.ap())
            b2col = cpool.tile([104, 1], f32)
            nc.sync.dma_start(b2col[:, 0], b2c_d.ap())
            b1col = cpool.tile([H1, 1], f32)
            nc.sync.dma_start(b1col[:, 0], b1_d.ap())
            a1rep = cpool.tile([H1, CH_ROWS], f16)
            nc.sync.dma_start(a1rep[:], a1r_d.ap())
            a2rep = cpool.tile([104, CH_ROWS], f16)
            nc.sync.dma_start(a2rep[:], a2r_d.ap())
            smegt = cpool.tile([104, 16, S2B], f16)
            nc.sync.dma_start(smegt[:], smeg_d.ap().rearrange("p (j c) -> p j c", c=S2B))
            zcol = cpool.tile([128, 1], f32)
            nc.vector.memset(zcol[:], 0.0)
            n8col = cpool.tile([S2B, 1], f32)
            nc.vector.memset(n8col[:], -8.0)

            # --- qT [64, BC] f32, C1T [H1, BC] = (q @ W1ac).T + b1 ---
            qT = cpool.tile([D, BC], f32)
            C1T = cpool.tile([H1, BC], f32)
            for g in range(2):
                qn = cpool.tile([128, D], f32, tag=f"qn{g}", name=f"qn{g}")
                nc.sync.dma_start(qn[:], q_d.ap()[g * 128:(g + 1) * 128, :])
                pt = pmpool.tile([128, 512], f32, tag="setup", name=f"ptq{g}")
                nc.tensor.transpose(pt[0:D, 0:128], qn[:], ident32[:])
                nc.vector.tensor_copy(qT[:, g * 128:(g + 1) * 128], pt[0:D, 0:128])
            for g in range(2):
                pt = pmpool.tile([128, 512], f32, tag="setup", name=f"ptc{g}")
                nc.tensor.matmul(pt[0:H1, 0:128], w1ac[:],
                                 qT[:, g * 128:(g + 1) * 128],
                                 start=True, stop=True)
                nc.vector.tensor_scalar(C1T[:, g * 128:(g + 1) * 128],
                                        pt[0:H1, 0:128], b1col[:], None, ALU.add)
            dC1T = cpool.tile([H1, BC // 2], f32)
            c1v = C1T[:].rearrange("p (c two) -> p c two", two=2)
            nc.vector.tensor_tensor(dC1T[:], c1v[:, :, 1], c1v[:, :, 0],
                                    ALU.subtract)

            # ======== main loop over super-chunks (32 batches) ========
            for s in range(N_S2):
                # ---- k: cast-load, transpose, dup, qk multiply ----
                knat = kbpool.tile([128, N_BLK * 128], f16, tag="knat")
                knv = knat[:].rearrange("p (n c) -> p n c", c=128)
                nc.gpsimd.dma_start(knv[:, :, 0:64],
                                    kv[:, s * N_BLK:(s + 1) * N_BLK, :])

                # ---- v: cast-load [t%100, b, t//100, d] ----
                vblk = vbpool.tile([100, S2B, 2, 64], f16, tag="vblk")
                nc.gpsimd.dma_start(
                    vblk[:],
                    v_d.ap()[s * S2B:(s + 1) * S2B, :, :]
                    .rearrange("b (h p) d -> p b h d", h=2))

                ktw = ktpool.tile([128, S2_ROWS], f16, tag="ktw")
                nc.sync.dma_start(
                    ktw[:].rearrange("p (n c) -> p n c", c=128),
                    knv[:], transpose=True)
                nc.sync.dma_start(ktw[64:128, :], ktw[0:64, :])
                if s % 2 == 0:
                    qrt2 = qrpool.tile([128, 2 * S2_ROWS], f16, tag="qrt")
                    nc.scalar.dma_start(
                        qrt2[64:128, :],
                        qrep_d.ap()[:, s * S2_ROWS:(s + 2) * S2_ROWS])
                qrt = qrt2[:, (s % 2) * S2_ROWS:(s % 2 + 1) * S2_ROWS]
                # qk multiply in halves (finer pipeline granularity)
                for hf in range(2):
                    sl = slice(hf * S2_ROWS // 2, (hf + 1) * S2_ROWS // 2)
                    nc.vector.tensor_tensor(ktw[64:128, sl], ktw[64:128, sl],
                                            qrt[64:128, sl], ALU.mult)

                # ---- mm1 + PReLU + mm2 + PReLU + logits ----
                pL = pLpool.tile([S2B, T], f32, tag="pL", name=f"pL{s}")
                for c2 in range(N_CH // 2):
                    h1pair = []
                    for cc in (2 * c2, 2 * c2 + 1):
                        p1 = p1pool.tile([H1, CH_ROWS], f32, tag="p1",
                                         name=f"p1_{s}_{cc}")
                        nc.tensor.matmul(p1[:], w1s16[:],
                                         ktw[:, cc * CH_ROWS:(cc + 1) * CH_ROWS],
                                         start=True, stop=True)
                        h1 = hpool.tile([H1, CH_ROWS], f16, tag="h1",
                                        name=f"h1_{s}_{cc}")
                        bg = s * S2B + cc * CH_B
                        nc.vector._custom_dve(
                            PRELU,
                            out=h1[:].rearrange("p (s n) -> p s n", s=2),
                            in0=p1[:].rearrange("p (s n) -> p s n", s=2),
                            in1=a1rep[:],
                            s0=C1T[:, bg:bg + 1],
                            s1=dC1T[:, bg // 2: bg // 2 + 1])
                        h1pair.append(h1)
                    p2 = p2pool.tile([104, CH_ROWS], f32, tag="p2",
                                     name=f"p2_{s}_{c2}")
                    nc.tensor.matmul(p2[0:H2, :], w2s16[:], h1pair[0][:],
                                     start=True, stop=True)
                    nc.tensor.matmul(p2[64:64 + H2, :], w2s16[:], h1pair[1][:],
                                     start=True, stop=True, tile_position=(0, 64))
                    h2 = hpool.tile([104, CH_ROWS], f16, tag="h2",
                                    name=f"h2_{s}_{c2}")
                    nc.vector._custom_dve(
                        PRELU,
                        out=h2[:].rearrange("p (s n) -> p s n", s=2),
                        in0=p2[:].rearrange("p (s n) -> p s n", s=2),
                        in1=a2rep[:],
                        s0=b2col[0:104], s1=zcol[0:104])
                    # logits for this pair: batches (4*c2+h) and (4*c2+2+h)
                    for h in range(2):
                        nc.tensor.matmul(
                            pL[:], smegt[:, 2 * c2 + h, :],
                            h2[:, T * h:T * (h + 1)],
                            start=(c2 == 0 and h == 0),
                            stop=(c2 == N_CH // 2 - 1 and h == 1))

                # ---- softmax (batch-major) ----
                u16 = smpool.tile([S2B, T], f16, tag="u16", name=f"u{s}")
                ssum = smpool.tile([S2B, 1], f32, tag="ssum", name=f"ss{s}")
                nc.scalar.activation(u16[:], pL[:], AF.Exp, bias=n8col[:],
                                     accum_out=ssum[:])
                rs = smpool.tile([S2B, 1], f32, tag="rs", name=f"rs{s}")
                nc.vector.reciprocal(rs[:], ssum[:])
                wts = smpool.tile([S2B, T], f16, tag="wts", name=f"w{s}")
                nc.vector.tensor_scalar(wts[:], u16[:], rs[:], None, ALU.mult)

                # ---- w.T via PE transpose: wT[t', 32h + b] ----
                psT = pmpool.tile([100, 64], f16, tag="setup", name=f"psT{s}")
                nc.tensor.transpose(psT[:, 0:S2B], wts[:, 0:100],
                                    ident16[0:S2B, 0:S2B])
                nc.tensor.transpose(psT[:, S2B:2 * S2B], wts[:, 100:200],
                                    ident16[0:S2B, 0:S2B])
                wT = smpool.tile([100, 64], f16, tag="wT", name=f"wT{s}")
                nc.vector.tensor_copy(wT[:], psT[:])

                # ---- w @ v: 8-batch groups; stationary = wT cols, moving =
                # v for 8 batches side by side; out[r, (b', d)] valid at
                # b' == r -> diagonal-extract DMA to HBM.
                ocp = smpool.tile([8, 4 * 512], f32, tag="ocp", name=f"ocp{s}")
                for g in range(S2B // 8):
                    pv = pvpool.tile([8, 512], f32, tag="pv", name=f"pv{s}_{g}")
                    for h in range(2):
                        nc.tensor.matmul(
                            pv[:].rearrange("r (b d) -> r b d", d=64),
                            wT[:, 32 * h + 8 * g:32 * h + 8 * g + 8],
                            vblk[:, 8 * g:8 * g + 8, h, :],
                            start=(h == 0), stop=(h == 1))
                    nc.scalar.copy(ocp[:, g * 512:(g + 1) * 512], pv[:])
                # diagonal extract, one SWDGE DMA per super-chunk:
                # src (r, g, d) at flat r*2112 + g*512 + d -> out row 8g+r
                oap = ocp[:]
                diag = AP(oap.tensor, oap.offset,
                          [[2112, 8], [512, 4], [1, 64]])
                nc.gpsimd.dma_start(
                    out_d.ap()[s * S2B:(s + 1) * S2B, :]
                    .rearrange("(g r) d -> r g d", r=8),
                    diag)

    nc.compile()
    _cache["nc"] = nc
    return nc


def _host_prep(W1, b1, a1, W2, b2, a2, Wf):
    """Host-side weight marshalling (tiny, one-time per call)."""
    f16 = np.float16
    w1s = np.ascontiguousarray(
        np.concatenate([W1[64:128] - W1[128:192], W1[192:256]], axis=0)
    ).astype(f16)                                          # [128, H1]
    w1ac = np.ascontiguousarray(W1[0:64] + W1[128:192], dtype=np.float32)
    b2c = np.zeros((104,), np.float32)
    b2c[0:H2] = b2
    b2c[64:64 + H2] = b2
    a1r = np.ascontiguousarray(np.tile(a1.T, (1, 2))).astype(f16)   # [80, 400]
    a2r = np.zeros((104, CH_ROWS), f16)
    a2T = np.tile(a2.T, (1, 2)).astype(f16)
    a2r[0:H2] = a2T
    a2r[64:64 + H2] = a2T
    smeg = np.zeros((104, 16 * S2B), f16)
    wfc = Wf[:, 0].astype(f16)
    for p in range(8):
        for h in range(2):
            j = 2 * p + h
            smeg[0:H2, j * S2B + 4 * p + h] = wfc
            smeg[64:64 + H2, j * S2B + 4 * p + 2 + h] = wfc
    return {
        "w1s": w1s, "w1ac": w1ac,
        "b1": np.ascontiguousarray(b1, dtype=np.float32),
        "w2s": np.ascontiguousarray(W2).astype(f16),
        "b2c": b2c, "a1r": a1r, "a2r": a2r, "smeg": smeg,
    }


def make_in_maps(q, k, v, W1, b1, a1, W2, b2, a2, Wf, bf):
    q = np.ascontiguousarray(np.asarray(q, dtype=np.float32))
    k = np.ascontiguousarray(np.asarray(k, dtype=np.float32))
    v = np.ascontiguousarray(np.asarray(v, dtype=np.float32))
    shared = _host_prep(np.asarray(W1, np.float32), np.asarray(b1, np.float32),
                        np.asarray(a1, np.float32), np.asarray(W2, np.float32),
                        np.asarray(b2, np.float32), np.asarray(a2, np.float32),
                        np.asarray(Wf, np.float32))
    in_maps = []
    for c in range(N_CORES):
        sl = slice(c * BC, (c + 1) * BC)
        qc = q[sl]
        # qrep [64, BC*T] f16: feature-major broadcast of q over t
        qrep = np.ascontiguousarray(
            np.broadcast_to(qc.T.astype(np.float16)[:, :, None],
                            (D, BC, T))).reshape(D, BC * T)
        m = {"q": qc, "k": k[sl], "v": v[sl], "qrep": qrep}
        m.update(shared)
        in_maps.append(m)
    return in_maps


def kernel(q, k, v, W1, b1, a1, W2, b2, a2, Wf, bf):
    from concourse.bass_utils import run_bass_kernel_spmd

    nc = _build()
    in_maps = make_in_maps(q, k, v, W1, b1, a1, W2, b2, a2, Wf, bf)
    res = run_bass_kernel_spmd(nc, in_maps, core_ids=list(range(N_CORES)))
    out = np.empty((B, D), dtype=np.float32)
    for c in range(N_CORES):
        out[c * BC:(c + 1) * BC] = res.results[c]["out"]
    return out


# revision 21
# speedup vs baseline: 1.1994x; 1.1994x over previous
"""Trainium2 Bass kernel for nn_AttentionLayer (DIN-style attention MLP).

Per batch row b (B=2048, T=200, D=64, H1=80, H2=40):
  info = [q, k, q-k, q*k];  y1 = info @ W1 + b1
       = k @ (W1b-W1c) + (q*k) @ W1d + (q @ (W1a+W1c) + b1)
  h1 = prelu(y1, a1);  y2 = h1 @ W2 + b2;  h2 = prelu(y2, a2)
  logits = h2 @ Wf;  w = softmax(logits);  out = w @ v

Pure data-parallel across 8 cores (256 batches each). v2 layout:
- 8 super-chunks of 32 batches (6400 rows) per core.
- k: SWDGE cast f32->f16 -> HWDGE xbar transpose (alternating SP/Act
  rings) -> SWDGE dup to partitions 64-127 -> ONE DVE tensor_tensor
  multiply against a host-marshalled broadcast of q (qrep, f16 input)
  forms (q*k).T in place. mm1 is a W-stationary f16 stream (N=400).
- PReLU: fused custom DVE op (select + per-batch bias via PageIdx).
- logits: Wf packed into zero-padded [104,32] stationaries so 16
  matmuls/super-chunk accumulate logits for all 32 batches into one
  [32,200] PSUM tile (batch-major).
- softmax: Act exp with fused row-sum accumulator, DVE reciprocal +
  scale; weights transposed to t-major via 2 PE transposes.
- w@v: w-stationary matmuls (LDWEIGHTS = 1 column) with
  tile_position col-quadrants packing 4 batches/PSUM-partition-group;
  one [128,512] PSUM bank holds all 32 outputs; Act copies to SBUF and
  a strided DMA writes [32,64] rows straight to HBM.
"""

import numpy as np

B, T, D = 2048, 200, 64
H1, H2 = 80, 40
N_CORES = 8
BC = B // N_CORES          # 256 batches per core
S2B = 32                   # batches per super-chunk
N_S2 = BC // S2B           # 8 super-chunks
S2_ROWS = S2B * T          # 6400 rows
N_BLK = S2_ROWS // 128     # 50 transpose blocks
CH_B = 2                   # batches per chunk
CH_ROWS = CH_B * T         # 400
N_CH = S2B // CH_B         # 16 chunks per super-chunk

_cache = {}


def _register_prelu_op():
    import concourse.dve_ops as dve_ops
    from concourse.dve_ops import DveOp, OPS
    from concourse.dve_spec import Spec, Src0, Src1, C0, C1, Zero, select, lower, PageIdx
    from concourse.dve_uop import DveOpSpec

    if "PRELU_PG_ANT" in dve_ops._SUB_OPCODE_FOR_NAME:
        for op in OPS:
            if op.name == "PRELU_PG_ANT":
                return op
    t = Src0 + PageIdx(C0, C1)
    spec = Spec(body=select(t > Zero, t, t * Src1))
    opcode = dve_ops._CUSTOM_DVE_ROW_BASE + len(OPS)
    shas = {}
    for ver in ("v3", "v4"):
        r = DveOpSpec(name="PRELU_PG_ANT", opcode=opcode,
                      uops=lower(spec, ver=ver), rd1_en=True)
        shas[ver] = r.sha(ver)
    op = DveOp("PRELU_PG_ANT", spec, subdim=True, uops_sha=shas)
    OPS.append(op)
    dve_ops._SUB_OPCODE_FOR_NAME["PRELU_PG_ANT"] = opcode
    return op


def _build():
    if "nc" in _cache:
        return _cache["nc"]
    import concourse.bacc as bacc
    import concourse.mybir as mybir
    import concourse.tile as tile
    from concourse import masks
    from concourse.ap import AP

    PRELU = _register_prelu_op()
    f16 = mybir.dt.float16
    f32 = mybir.dt.float32
    AF = mybir.ActivationFunctionType
    ALU = mybir.AluOpType

    nc = bacc.Bacc("TRN2", target_bir_lowering=False, debug=False,
                   num_devices=N_CORES)

    q_d = nc.dram_tensor("q", [BC, D], f32, kind="ExternalInput")
    k_d = nc.dram_tensor("k", [BC, T, D], f32, kind="ExternalInput")
    v_d = nc.dram_tensor("v", [BC, T, D], f32, kind="ExternalInput")
    qrep_d = nc.dram_tensor("qrep", [D, BC * T], f16, kind="ExternalInput")
    w1s_d = nc.dram_tensor("w1s", [128, H1], f16, kind="ExternalInput")
    w1ac_d = nc.dram_tensor("w1ac", [D, H1], f32, kind="ExternalInput")
    b1_d = nc.dram_tensor("b1", [H1], f32, kind="ExternalInput")
    w2s_d = nc.dram_tensor("w2s", [H1, H2], f16, kind="ExternalInput")
    b2c_d = nc.dram_tensor("b2c", [104], f32, kind="ExternalInput")
    a1r_d = nc.dram_tensor("a1r", [H1, CH_ROWS], f16, kind="ExternalInput")
    a2r_d = nc.dram_tensor("a2r", [104, CH_ROWS], f16, kind="ExternalInput")
    smeg_d = nc.dram_tensor("smeg", [104, 16 * S2B], f16, kind="ExternalInput")
    out_d = nc.dram_tensor("out", [BC, D], f32, kind="ExternalOutput")

    kv = k_d.ap().rearrange("b t d -> (b t) d").rearrange("(n p) d -> p n d", p=128)

    with tile.TileContext(nc) as tc:
        with (
            tc.tile_pool(name="const", bufs=1) as cpool,
            tc.tile_pool(name="kb", bufs=2) as kbpool,
            tc.tile_pool(name="kt", bufs=2) as ktpool,
            tc.tile_pool(name="qr", bufs=2) as qrpool,
            tc.tile_pool(name="vb", bufs=2) as vbpool,
            tc.tile_pool(name="h", bufs=4) as hpool,
            tc.tile_pool(name="sm", bufs=2) as smpool,
            tc.tile_pool(name="p1", bufs=2, space="PSUM") as p1pool,
            tc.tile_pool(name="p2", bufs=2, space="PSUM") as p2pool,
            tc.tile_pool(name="pL", bufs=1, space="PSUM") as pLpool,
            tc.tile_pool(name="pv", bufs=2, space="PSUM") as pvpool,
            tc.tile_pool(name="pm", bufs=1, space="PSUM") as pmpool,
        ):
            # ======== one-time setup ========
            ident32 = cpool.tile([128, 128], f32)
            masks.make_identity(nc, ident32[:])
            ident16 = cpool.tile([64, 64], f16)
            nc.vector.tensor_copy(ident16[:], ident32[0:64, 0:64])

            w1s16 = cpool.tile([128, H1], f16)
            nc.sync.dma_start(w1s16[:], w1s_d.ap())
            w1ac = cpool.tile([D, H1], f32)
            nc.sync.dma_start(w1ac[:], w1ac_d.ap())
            w2s16 = cpool.tile([H1, H2], f16)
            nc.sync.dma_start(w2s16[:], w2s_d.ap())
            b2col = cpool.tile([104, 1], f32)
            nc.sync.dma_start(b2col[:, 0], b2c_d.ap())
            b1col = cpool.tile([H1, 1], f32)
            nc.sync.dma_start(b1col[:, 0], b1_d.ap())
            a1rep = cpool.tile([H1, CH_ROWS], f16)
            nc.sync.dma_start(a1rep[:], a1r_d.ap())
            a2rep = cpool.tile([104, CH_ROWS], f16)
            nc.sync.dma_start(a2rep[:], a2r_d.ap())
            smegt = cpool.tile([104, 16, S2B], f16)
            nc.sync.dma_start(smegt[:], smeg_d.ap().rearrange("p (j c) -> p j c", c=S2B))
            zcol = cpool.tile([128, 1], f32)
            nc.vector.memset(zcol[:], 0.0)
            n8col = cpool.tile([S2B, 1], f32)
            nc.vector.memset(n8col[:], -8.0)

            # --- qT [64, BC] f32, C1T [H1, BC] = (q @ W1ac).T + b1 ---
            qT = cpool.tile([D, BC], f32)
            C1T = cpool.tile([H1, BC], f32)
            for g in range(2):
                qn = cpool.tile([128, D], f32, tag=f"qn{g}", name=f"qn{g}")
                nc.sync.dma_start(qn[:], q_d.ap()[g * 128:(g + 1) * 128, :])
                pt = pmpool.tile([128, 512], f32, tag="setup", name=f"ptq{g}")
                nc.tensor.transpose(pt[0:D, 0:128], qn[:], ident32[:])
                nc.vector.tensor_copy(qT[:, g * 128:(g + 1) * 128], pt[0:D, 0:128])
            for g in range(2):
                pt = pmpool.tile([128, 512], f32, tag="setup", name=f"ptc{g}")
                nc.tensor.matmul(pt[0:H1, 0:128], w1ac[:],
                                 qT[:, g * 128:(g + 1) * 128],
                                 start=True, stop=True)
                nc.vector.tensor_scalar(C1T[:, g * 128:(g + 1) * 128],
                                        pt[0:H1, 0:128], b1col[:], None, ALU.add)
            dC1T = cpool.tile([H1, BC // 2], f32)
            c1v = C1T[:].rearrange("p (c two) -> p c two", two=2)
            nc.vector.tensor_tensor(dC1T[:], c1v[:, :, 1], c1v[:, :, 0],
                                    ALU.subtract)

            # ======== main loop over super-chunks (32 batches) ========
            for s in range(N_S2):
                # ---- k: cast-load, transpose, dup, qk multiply ----
                knat = kbpool.tile([128, N_BLK * 128], f16, tag="knat")
                knv = knat[:].rearrange("p (n c) -> p n c", c=128)
                nc.gpsimd.dma_start(knv[:, :, 0:64],
                                    kv[:, s * N_BLK:(s + 1) * N_BLK, :])

                # ---- v: cast-load [t%100, b, t//100, d] ----
                vblk = vbpool.tile([100, S2B, 2, 64], f16, tag="vblk")
                nc.gpsimd.dma_start(
                    vblk[:],
                    v_d.ap()[s * S2B:(s + 1) * S2B, :, :]
                    .rearrange("b (h p) d -> p b h d", h=2))

                ktw = ktpool.tile([128, S2_ROWS], f16, tag="ktw")
                nc.sync.dma_start(
                    ktw[:].rearrange("p (n c) -> p n c", c=128),
                    knv[:], transpose=True)
                nc.sync.dma_start(ktw[64:128, :], ktw[0:64, :])
                if s % 2 == 0:
                    qrt2 = qrpool.tile([128, 2 * S2_ROWS], f16, tag="qrt")
                    nc.scalar.dma_start(
                        qrt2[64:128, :],
                        qrep_d.ap()[:, s * S2_ROWS:(s + 2) * S2_ROWS])
                qrt = qrt2[:, (s % 2) * S2_ROWS:(s % 2 + 1) * S2_ROWS]
                # qk multiply in halves (finer pipeline granularity)
                for hf in range(2):
                    sl = slice(hf * S2_ROWS // 2, (hf + 1) * S2_ROWS // 2)
                    nc.vector.tensor_tensor(ktw[64:128, sl], ktw[64:128, sl],
                                            qrt[64:128, sl], ALU.mult)

                # ---- mm1 + PReLU + mm2 + PReLU + logits ----
                pL = pLpool.tile([S2B, T], f32, tag="pL", name=f"pL{s}")
                for c2 in range(N_CH // 2):
                    h1pair = []
                    for cc in (2 * c2, 2 * c2 + 1):
                        p1 = p1pool.tile([H1, CH_ROWS], f32, tag="p1",
                                         name=f"p1_{s}_{cc}")
                        nc.tensor.matmul(p1[:], w1s16[:],
                                         ktw[:, cc * CH_ROWS:(cc + 1) * CH_ROWS],
                                         start=True, stop=True)
                        h1 = hpool.tile([H1, CH_ROWS], f16, tag="h1",
                                        name=f"h1_{s}_{cc}")
                        bg = s * S2B + cc * CH_B
                        nc.vector._custom_dve(
                            PRELU,
                            out=h1[:].rearrange("p (s n) -> p s n", s=2),
                            in0=p1[:].rearrange("p (s n) -> p s n", s=2),
                            in1=a1rep[:],
                            s0=C1T[:, bg:bg + 1],
                            s1=dC1T[:, bg // 2: bg // 2 + 1])
                        h1pair.append(h1)
                    p2 = p2pool.tile([104, CH_ROWS], f32, tag="p2",
                                     name=f"p2_{s}_{c2}")
                    nc.tensor.matmul(p2[0:H2, :], w2s16[:], h1pair[0][:],
                                     start=True, stop=True)
                    nc.tensor.matmul(p2[64:64 + H2, :], w2s16[:], h1pair[1][:],
                                     start=True, stop=True, tile_position=(0, 64))
                    h2 = hpool.tile([104, CH_ROWS], f16, tag="h2",
                                    name=f"h2_{s}_{c2}")
                    nc.vector._custom_dve(
                        PRELU,
                        out=h2[:].rearrange("p (s n) -> p s n", s=2),
                        in0=p2[:].rearrange("p (s n) -> p s n", s=2),
                        in1=a2rep[:],
                        s0=b2col[0:104], s1=zcol[0:104])
                    # logits for this pair: batches (4*c2+h) and (4*c2+2+h)
                    for h in range(2):
                        nc.tensor.matmul(
                            pL[:], smegt[:, 2 * c2 + h, :],
                            h2[:, T * h:T * (h + 1)],
                            start=(c2 == 0 and h == 0),
                            stop=(c2 == N_CH // 2 - 1 and h == 1))

                # ---- softmax (batch-major) ----
                u16 = smpool.tile([S2B, T], f16, tag="u16", name=f"u{s}")
                ssum = smpool.tile([S2B, 1], f32, tag="ssum", name=f"ss{s}")
                nc.scalar.activation(u16[:], pL[:], AF.Exp, bias=n8col[:],
                                     accum_out=ssum[:])
                rs = smpool.tile([S2B, 1], f32, tag="rs", name=f"rs{s}")
                nc.vector.reciprocal(rs[:], ssum[:])
                wts = smpool.tile([S2B, T], f16, tag="wts", name=f"w{s}")
                nc.vector.tensor_scalar(wts[:], u16[:], rs[:], None, ALU.mult)

                # ---- w.T via PE transpose: wT[t', 32h + b] ----
                psT = pmpool.tile([100, 64], f16, tag="setup", name=f"psT{s}")
                nc.tensor.transpose(psT[:, 0:S2B], wts[:, 0:100],
                                    ident16[0:S2B, 0:S2B])
                nc.tensor.transpose(psT[:, S2B:2 * S2B], wts[:, 100:200],
                                    ident16[0:S2B, 0:S2B])
                wT = smpool.tile([100, 64], f16, tag="wT", name=f"wT{s}")
                nc.vector.tensor_copy(wT[:], psT[:])

                # ---- w @ v: 8-batch groups; stationary = wT cols, moving =
                # v for 8 batches side by side; out[r, (b', d)] valid at
                # b' == r -> diagonal-extract DMA to HBM.
                ocp = smpool.tile([8, 4 * 512], f32, tag="ocp", name=f"ocp{s}")
                for g in range(S2B // 8):
                    pv = pvpool.tile([8, 512], f32, tag="pv", name=f"pv{s}_{g}")
                    for h in range(2):
                        nc.tensor.matmul(
                            pv[:].rearrange("r (b d) -> r b d", d=64),
                            wT[:, 32 * h + 8 * g:32 * h + 8 * g + 8],
                            vblk[:, 8 * g:8 * g + 8, h, :],
                            start=(h == 0), stop=(h == 1))
                    nc.scalar.copy(ocp[:, g * 512:(g + 1) * 512], pv[:])
                # diagonal extract, one SWDGE DMA per super-chunk:
                # src (r, g, d) at flat r*2112 + g*512 + d -> out row 8g+r
                oap = ocp[:]
                diag = AP(oap.tensor, oap.offset,
                          [[2112, 8], [512, 4], [1, 64]])
                nc.sync.dma_start(
                    out_d.ap()[s * S2B:(s + 1) * S2B, :]
                    .rearrange("(g r) d -> r g d", r=8),
                    diag)

    nc.compile()
    _cache["nc"] = nc
    return nc


def _host_prep(W1, b1, a1, W2, b2, a2, Wf):
    """Host-side weight marshalling (tiny, one-time per call)."""
    f16 = np.float16
    w1s = np.ascontiguousarray(
        np.concatenate([W1[64:128] - W1[128:192], W1[192:256]], axis=0)
    ).astype(f16)                                          # [128, H1]
    w1ac = np.ascontiguousarray(W1[0:64] + W1[128:192], dtype=np.float32)
    b2c = np.zeros((104,), np.float32)
    b2c[0:H2] = b2
    b2c[64:64 + H2] = b2
    a1r = np.ascontiguousarray(np.tile(a1.T, (1, 2))).astype(f16)   # [80, 400]
    a2r = np.zeros((104, CH_ROWS), f16)
    a2T = np.tile(a2.T, (1, 2)).astype(f16)
    a2r[0:H2] = a2T
    a2r[64:64 + H2] = a2T
    smeg = np.zeros((104, 16 * S2B), f16)
    wfc = Wf[:, 0].astype(f16)
    for p in range(8):
        for h in range(2):
            j = 2 * p + h
            smeg[0:H2, j * S2B + 4 * p + h] = wfc
            smeg[64:64 + H2, j * S2B + 4 * p + 2 + h] = wfc
    return {
        "w1s": w1s, "w1ac": w1ac,
        "b1": np.ascontiguousarray(b1, dtype=np.float32),
        "w2s": np.ascontiguousarray(W2).astype(f16),
        "b2c": b2c, "a1r": a1r, "a2r": a2r, "smeg": smeg,
    }


def make_in_maps(q, k, v, W1, b1, a1, W2, b2, a2, Wf, bf):
    q = np.ascontiguousarray(np.asarray(q, dtype=np.float32))
    k = np.ascontiguousarray(np.asarray(k, dtype=np.float32))
    v = np.ascontiguousarray(np.asarray(v, dtype=np.float32))
    shared = _host_prep(np.asarray(W1, np.float32), np.asarray(b1, np.float32),
                        np.asarray(a1, np.float32), np.asarray(W2, np.float32),
                        np.asarray(b2, np.float32), np.asarray(a2, np.float32),
                        np.asarray(Wf, np.float32))
    in_maps = []
    for c in range(N_CORES):
        sl = slice(c * BC, (c + 1) * BC)
        qc = q[sl]
        # qrep [64, BC*T] f16: feature-major broadcast of q over t
        qrep = np.ascontiguousarray(
            np.broadcast_to(qc.T.astype(np.float16)[:, :, None],
                            (D, BC, T))).reshape(D, BC * T)
        m = {"q": qc, "k": k[sl], "v": v[sl], "qrep": qrep}
        m.update(shared)
        in_maps.append(m)
    return in_maps


def kernel(q, k, v, W1, b1, a1, W2, b2, a2, Wf, bf):
    from concourse.bass_utils import run_bass_kernel_spmd

    nc = _build()
    in_maps = make_in_maps(q, k, v, W1, b1, a1, W2, b2, a2, Wf, bf)
    res = run_bass_kernel_spmd(nc, in_maps, core_ids=list(range(N_CORES)))
    out = np.empty((B, D), dtype=np.float32)
    for c in range(N_CORES):
        out[c * BC:(c + 1) * BC] = res.results[c]["out"]
    return out
